# revision 30
# baseline (speedup 1.0000x reference)
"""Trainium2 Bass kernel for HadamardTernaryLinear.

y = reshape( (FHT_g(x*alpha) @grouped w_q) -> FHT_h -> *beta ), with
w_q = BitNet-style absmean ternary quantization of weight.

Strategy: data-parallel over the 8192 tokens across 8 NeuronCores (1024
tokens/core, no collectives). Per core, a 3-pass TensorEngine pipeline in
bf16 (Hadamard and ternary weights are exact in bf16; quant scale, alpha
and beta fold into f32 host prep / the final drain):

  P1 FHT_g : MM(lhsT=I4(x)H32, rhs=xa_k)   32x N=512  -> xm [(i',h), tok]
  R1       : regroup via DRAM bounce       -> xb [i, tok] (h-grouped)
  P2 GM    : MM(lhsT=WqT_h,    rhs=xb_h)   32x N=512  -> yp [(o',m), tok]
  R2       : same regroup pattern          -> ya [(h,o'), tok]
  P3 FHT_h : MM(lhsT=H32(x)I4, rhs=ya_m)   32x N=512  -> [(g,o'), tok]
  drain P3 on Scalar/Vector with per-row scale beta*quantscale/32, bf16.

Both regroups are the partition permutation dst[4c+a, b*512+t] =
src[a*32+b, c*512+t] (R2 reuses it because the weight columns are
permuted host-side so P2's output partition index is o'*32+m, o=4m+o').
Pure SBUF->SBUF forms of this permutation either fail the BIR verifier
(partition step 4) or crash the runtime DGE (partition step 32), and
per-k 4-partition-destination DMAs bottleneck on one SDMA port per
queue ring.  The DRAM bounce sidesteps all of that: 8 thin writes lay
the tile out as dram[4c+a, (b,t)], then 2 fat reads (128 rows x 16KB)
land it in SBUF; every hop is a plain partition-step-1 pattern using
all 16 SDMA ports, split across the SP/ACT HWDGE and Pool SWDGE rings.

PSUM->SBUF drains rotate across Vector/Scalar (GpSimd cannot read PSUM
on trn2).  Input arrives pre-transposed from host (feature-major,
supertile-blocked) so input/output DMAs are fully linear (128 x 32KB).
"""

import functools
import sys

for _p in ("/opt/trn_rl_repo",):
    if _p not in sys.path:
        sys.path.insert(0, _p)

import ml_dtypes
import numpy as np

import concourse.mybir as mybir
import concourse.tile as tile
from concourse import bacc
from concourse.bass_utils import run_bass_kernel_spmd

G = 32
IO = 128  # in_o
OO = 128  # out_o
D = G * IO  # 4096
NCORES = 8
B, T = 4, 2048
BT = B * T
TOKC = BT // NCORES  # tokens per core
ST = 512  # supertile tokens
NST = TOKC // ST
KT = D // 128  # 32 feature tiles

DTB = mybir.dt.bfloat16
DTF = mybir.dt.float32
BF16 = ml_dtypes.bfloat16


def _hadamard(n):
    H = np.array([[1.0]], dtype=np.float32)
    while H.shape[0] < n:
        H = np.block([[H, H], [H, -H]])
    return H  # +-1, symmetric


def build_body(nc, tc, xin, hm, wqm, bc, yout, xmd, ypd, loop_r=1):
    """Emit the per-core program, software-pipelined across supertiles so
    each strict-FIFO engine queue sees instructions in readiness order
    (P1 of supertile n+1 is emitted before P3 of supertile n, which waits
    on a DRAM-bounce round trip)."""

    with (
        tc.tile_pool(name="const", bufs=1) as cpool,
        tc.tile_pool(name="xa", bufs=2) as xapool,
        tc.tile_pool(name="mid", bufs=1) as mpool,
        tc.tile_pool(name="ps1", bufs=3, space="PSUM") as ps1pool,
        tc.tile_pool(name="ps2", bufs=3, space="PSUM") as ps2pool,
        tc.tile_pool(name="ps3", bufs=2, space="PSUM") as ps3pool,
    ):
        hmt = cpool.tile([128, 128], DTB, tag="hm")
        nc.scalar.dma_start(hmt[:], hm[0])
        hmt2 = cpool.tile([128, 128], DTB, tag="hm2")
        nc.scalar.dma_start(hmt2[:], hm[1])
        wqt = cpool.tile([128, G * OO], DTB, tag="wq")
        nc.gpsimd.dma_start(wqt[:], wqm[:])
        bct = cpool.tile([128, G], DTF, tag="bc")
        nc.gpsimd.dma_start(bct[:], bc[:])

        QF = KT * ST // 4
        iengs = (nc.sync, nc.scalar, nc.gpsimd, nc.sync)

        # GPSIMD cannot read PSUM on trn2 — drains go Vector/Scalar only.
        def drain(idx, o, i):
            if idx % 8 in (1, 4, 6):
                nc.scalar.copy(o, i)
            else:
                nc.vector.tensor_copy(o, i)

        regroup_seq = [0]
        XA, XM, XB, YP, YA = {}, {}, {}, {}, {}

        def regroup(dst, src, dtmp):
            # dst[4c+a, b*512+t] = src[a*32+b, c*512+t] via DRAM bounce:
            # 16 thin writes (c-quarters) lay the tile out as dram[4c+a,
            # (b,t)], 4 fat reads land it in SBUF.  All hops are plain
            # partition-step-1 patterns using all 16 SDMA ports.
            engs = (nc.sync, nc.scalar, nc.gpsimd)
            dv = dtmp.rearrange("(c i) (b t) -> i b c t", c=KT, b=KT)
            QW = KT // 4
            for ip in range(4):
                s = src[ip * 32 : (ip + 1) * 32, :].rearrange(
                    "b (c t) -> b c t", c=KT
                )
                for ch in range(4):
                    eng = engs[regroup_seq[0] % 3]
                    regroup_seq[0] += 1
                    eng.dma_start(
                        dv[ip][:, ch * QW : (ch + 1) * QW],
                        s[:, ch * QW : (ch + 1) * QW],
                    )
            QC = KT // 4
            for bh in range(4):
                eng = engs[regroup_seq[0] % 3]
                regroup_seq[0] += 1
                fs = bh * QC * ST
                eng.dma_start(dst[:, fs : fs + QC * ST], dtmp[:, fs : fs + QC * ST])

        def emit_load(st):
            xa = xapool.tile([128, KT * ST], DTB, tag="xa")
            for q in range(4):
                iengs[q].dma_start(
                    xa[:, q * QF : (q + 1) * QF], xin[st][:, q * QF : (q + 1) * QF]
                )
            XA[st] = xa

        def emit_p1(st):
            xm = mpool.tile([128, KT * ST], DTB, tag="m1")
            for k in range(KT):
                ps = ps1pool.tile([128, ST], DTF, tag="ps1")
                nc.tensor.matmul(
                    ps[:], lhsT=hmt[:], rhs=XA[st][:, k * ST : (k + 1) * ST],
                    start=True, stop=True,
                )
                drain(k, xm[:, k * ST : (k + 1) * ST], ps[:])
            XM[st] = xm

        def emit_r1(st):
            xb = mpool.tile([128, KT * ST], DTB, tag="m2")
            regroup(xb, XM[st], xmd[st])
            XB[st] = xb

        def emit_p2(st):
            yp = mpool.tile([128, KT * ST], DTB, tag="m3")
            for h in range(KT):
                ps = ps2pool.tile([128, ST], DTF, tag="ps2")
                nc.tensor.matmul(
                    ps[:], lhsT=wqt[:, h * 128 : (h + 1) * 128],
                    rhs=XB[st][:, h * ST : (h + 1) * ST],
                    start=True, stop=True,
                )
                drain(h + 1, yp[:, h * ST : (h + 1) * ST], ps[:])
            YP[st] = yp

        def emit_r2(st):
            # ya reuses the xa pool rotation: the xa buffer of this
            # supertile is dead once P1 consumed it.
            ya = xapool.tile([128, KT * ST], DTB, tag="xa")
            regroup(ya, YP[st], ypd[st])
            YA[st] = ya

        def emit_p3(st):
            yf = mpool.tile([128, KT * ST], DTB, tag="m5")
            for m in range(KT):
                ps = ps3pool.tile([128, ST], DTF, tag="ps3")
                nc.tensor.matmul(
                    ps[:], lhsT=hmt2[:], rhs=YA[st][:, m * ST : (m + 1) * ST],
                    start=True, stop=True,
                )
                if m % 8 in (1, 4, 6):
                    nc.scalar.activation(
                        yf[:, m * ST : (m + 1) * ST], ps[:],
                        mybir.ActivationFunctionType.Copy,
                        scale=bct[:, m : m + 1],
                    )
                else:
                    nc.vector.tensor_scalar_mul(
                        yf[:, m * ST : (m + 1) * ST], ps[:], bct[:, m : m + 1]
                    )
            for q in range(4):
                iengs[q].dma_start(
                    yout[st][:, q * QF : (q + 1) * QF], yf[:, q * QF : (q + 1) * QF]
                )

        def body():
            emit_load(0)
            emit_p1(0)
            emit_r1(0)
            emit_load(1)
            emit_p1(1)
            emit_p2(0)
            emit_r1(1)
            emit_r2(0)
            emit_p2(1)
            emit_p3(0)
            emit_r2(1)
            emit_p3(1)

        if loop_r == 1:
            body()
        else:
            with tc.For_i(0, loop_r, 1):
                body()


@functools.lru_cache(maxsize=4)
def build_program(loop_r=1):
    nc = bacc.Bacc("TRN2", target_bir_lowering=False, debug=False)
    xin = nc.dram_tensor("xin", [NST, 128, KT * ST], DTB, kind="ExternalInput").ap()
    hm = nc.dram_tensor("hmat", [2, 128, 128], DTB, kind="ExternalInput").ap()
    wqm = nc.dram_tensor("wqm", [128, G * OO], DTB, kind="ExternalInput").ap()
    bc = nc.dram_tensor("betacol", [128, G], DTF, kind="ExternalInput").ap()
    yout = nc.dram_tensor("yout", [NST, 128, KT * ST], DTB, kind="ExternalOutput").ap()
    xmd = nc.dram_tensor("xmd", [NST, 128, KT * ST], DTB, kind="Internal").ap()
    ypd = nc.dram_tensor("ypd", [NST, 128, KT * ST], DTB, kind="Internal").ap()
    with tile.TileContext(nc) as tc:
        build_body(nc, tc, xin, hm, wqm, bc, yout, xmd, ypd, loop_r=loop_r)
    nc.compile()
    return nc


def host_prep(x, weight, alpha, beta):
    """Returns per-core input maps. Pure f32 numpy glue + bf16 casts."""
    H = _hadamard(G)  # [g,h] +-1

    w = np.asarray(weight, dtype=np.float32)
    scale = np.float32(np.mean(np.abs(w))) + np.float32(1e-8)
    wq3 = np.clip(np.round(w / scale), -1.0, 1.0).astype(np.float32)  # [h,o,i]

    # x * alpha, feature order f' = i*32+g (i-major)
    xp = np.asarray(x, dtype=np.float32).reshape(BT, G, IO) * np.asarray(
        alpha, dtype=np.float32
    )[None]
    xp = np.ascontiguousarray(xp.transpose(0, 2, 1)).reshape(BT, D)  # [tok, f']
    # device layout: [core, st, p(128), k(32), t(512)] with f' = k*128 + p
    xin_all = np.ascontiguousarray(
        xp.reshape(NCORES, NST, ST, KT, 128).transpose(0, 1, 4, 3, 2)
    ).reshape(NCORES, NST, 128, KT * ST).astype(BF16)

    hmat = np.stack(
        [
            np.kron(np.eye(4, dtype=np.float32), H),  # P1: I4 (x) H
            np.kron(H, np.eye(4, dtype=np.float32)),  # P3: H (x) I4
        ]
    ).astype(BF16)
    # wq_sb[i, h*128 + o'*32 + m] = wq3[h, 4m+o', i]  (o = 4m+o')
    wq_sb = np.ascontiguousarray(
        wq3.reshape(G, G, 4, IO).transpose(3, 0, 2, 1)
    ).reshape(IO, G * OO).astype(BF16)

    beta_f = np.asarray(beta, dtype=np.float32) * (scale / np.float32(G))  # [g,o]
    # betacol[p = 4g+o', m] = beta_f[g, 4m+o']
    bct = np.ascontiguousarray(
        beta_f.reshape(G, G, 4).transpose(0, 2, 1)
    ).reshape(128, G).astype(np.float32)

    in_maps = []
    for c in range(NCORES):
        in_maps.append(
            {
                "xin": xin_all[c],
                "hmat": hmat,
                "wqm": wq_sb,
                "betacol": bct,
            }
        )
    return in_maps


def host_post(results):
    ydev = np.stack([r["yout"] for r in results])  # [8, NST, 128, 16384] bf16
    # [c, st, p=(4g+o'), m*512+t] -> y[tok, g*128+4m+o']
    y = ydev.reshape(NCORES, NST, G, 4, KT, ST)  # [c, st, g, o', m, t]
    y = y.transpose(0, 1, 5, 2, 4, 3)  # [c, st, t, g, m, o']
    y = np.ascontiguousarray(y).astype(np.float32).reshape(BT, D)
    return y.reshape(B, T, D)


def kernel(x, weight, alpha, beta):
    nc = build_program(loop_r=1)
    in_maps = host_prep(x, weight, alpha, beta)
    res = run_bass_kernel_spmd(nc, in_maps, core_ids=list(range(NCORES)))
    return host_post(res.results)


# revision 32
# speedup vs baseline: 1.1065x; 1.1065x over previous
"""Trainium2 Bass kernel for HadamardTernaryLinear.

y = reshape( (FHT_g(x*alpha) @grouped w_q) -> FHT_h -> *beta ), with
w_q = BitNet-style absmean ternary quantization of weight.

Strategy: data-parallel over the 8192 tokens across 8 NeuronCores (1024
tokens/core, no collectives). Per core, a 3-pass TensorEngine pipeline in
bf16 (Hadamard and ternary weights are exact in bf16; quant scale, alpha
and beta fold into f32 host prep / the final drain):

  P1 FHT_g : MM(lhsT=I4(x)H32, rhs=xa_k)   32x N=512  -> xm [(i',h), tok]
  R1       : regroup via DRAM bounce       -> xb [i, tok] (h-grouped)
  P2 GM    : MM(lhsT=WqT_h,    rhs=xb_h)   32x N=512  -> yp [(o',m), tok]
  R2       : same regroup pattern          -> ya [(h,o'), tok]
  P3 FHT_h : MM(lhsT=H32(x)I4, rhs=ya_m)   32x N=512  -> [(g,o'), tok]
  drain P3 on Scalar/Vector with per-row scale beta*quantscale/32, bf16.

Both regroups are the partition permutation dst[4c+a, b*512+t] =
src[a*32+b, c*512+t] (R2 reuses it because the weight columns are
permuted host-side so P2's output partition index is o'*32+m, o=4m+o').
Pure SBUF->SBUF forms of this permutation either fail the BIR verifier
(partition step 4) or crash the runtime DGE (partition step 32), and
per-k 4-partition-destination DMAs bottleneck on one SDMA port per
queue ring.  The DRAM bounce sidesteps all of that: 8 thin writes lay
the tile out as dram[4c+a, (b,t)], then 2 fat reads (128 rows x 16KB)
land it in SBUF; every hop is a plain partition-step-1 pattern using
all 16 SDMA ports, split across the SP/ACT HWDGE and Pool SWDGE rings.

PSUM->SBUF drains rotate across Vector/Scalar (GpSimd cannot read PSUM
on trn2).  Input arrives pre-transposed from host (feature-major,
supertile-blocked) so input/output DMAs are fully linear (128 x 32KB).
"""

import functools
import sys

for _p in ("/opt/trn_rl_repo",):
    if _p not in sys.path:
        sys.path.insert(0, _p)

import ml_dtypes
import numpy as np

import concourse.mybir as mybir
import concourse.tile as tile
from concourse import bacc
from concourse.bass_utils import run_bass_kernel_spmd

G = 32
IO = 128  # in_o
OO = 128  # out_o
D = G * IO  # 4096
NCORES = 8
B, T = 4, 2048
BT = B * T
TOKC = BT // NCORES  # tokens per core
ST = 512  # supertile tokens
NST = TOKC // ST
KT = D // 128  # 32 feature tiles

DTB = mybir.dt.bfloat16
DTF = mybir.dt.float32
BF16 = ml_dtypes.bfloat16


def _hadamard(n):
    H = np.array([[1.0]], dtype=np.float32)
    while H.shape[0] < n:
        H = np.block([[H, H], [H, -H]])
    return H  # +-1, symmetric


def build_body(nc, tc, xin, hm, wqm, bc, yout, xmd, ypd, loop_r=1):
    """Emit the per-core program. All APs are DRAM tensors."""

    with (
        tc.tile_pool(name="const", bufs=1) as cpool,
        tc.tile_pool(name="xa", bufs=2) as xapool,
        tc.tile_pool(name="mid", bufs=1) as mpool,
        tc.tile_pool(name="ps1", bufs=3, space="PSUM") as ps1pool,
        tc.tile_pool(name="ps2", bufs=3, space="PSUM") as ps2pool,
        tc.tile_pool(name="ps3", bufs=2, space="PSUM") as ps3pool,
    ):
        hmt = cpool.tile([128, 128], DTB, tag="hm")
        nc.scalar.dma_start(hmt[:], hm[0])
        hmt2 = cpool.tile([128, 128], DTB, tag="hm2")
        nc.scalar.dma_start(hmt2[:], hm[1])
        wqt = cpool.tile([128, G * OO], DTB, tag="wq")
        nc.gpsimd.dma_start(wqt[:], wqm[:])
        bct = cpool.tile([128, G], DTF, tag="bc")
        nc.gpsimd.dma_start(bct[:], bc[:])

        # GPSIMD cannot read PSUM on trn2 — drains go Vector/Scalar only.
        def drain(idx, o, i):
            if idx % 2 == 1:
                nc.scalar.copy(o, i)
            else:
                nc.vector.tensor_copy(o, i)

        # Regroup dst[4c+a, b*512+t] = src[a*32+b, c*512+t] via a DRAM
        # bounce: 4 thin writes lay the data out as dram[4c+a, (b,t)], then
        # one fat read (128 rows x 32KB) lands it in SBUF.  All APs are
        # plain partition-step-1 patterns (runtime-safe), and both hops use
        # all 16 SDMA ports.
        regroup_seq = [0]

        def regroup(dst, src, dtmp):
            engs = (nc.sync, nc.scalar, nc.gpsimd)
            dv = dtmp.rearrange("(c i) (b t) -> i b c t", c=KT, b=KT)
            HC = KT // 2
            QW = KT // 4
            for ip in range(4):
                s = src[ip * 32 : (ip + 1) * 32, :].rearrange(
                    "b (c t) -> b c t", c=KT
                )
                for ch in range(4):
                    eng = engs[regroup_seq[0] % 3]
                    regroup_seq[0] += 1
                    eng.dma_start(
                        dv[ip][:, ch * QW : (ch + 1) * QW],
                        s[:, ch * QW : (ch + 1) * QW],
                    )
            QC = KT // 4
            fengs = (nc.sync, nc.gpsimd)
            for bh in range(4):
                eng = fengs[regroup_seq[0] % 2]
                regroup_seq[0] += 1
                fs = bh * QC * ST
                eng.dma_start(dst[:, fs : fs + QC * ST], dtmp[:, fs : fs + QC * ST])

        def supertile(st):
            # ---- load: fully linear (host pre-transposed, supertile-blocked)
            xa = xapool.tile([128, KT * ST], DTB, tag="xa")
            HF = KT * ST // 2
            QF = KT * ST // 4
            iengs = (nc.sync, nc.scalar, nc.gpsimd, nc.sync)
            for q in range(4):
                iengs[q].dma_start(
                    xa[:, q * QF : (q + 1) * QF], xin[st][:, q * QF : (q + 1) * QF]
                )

            # ---- P1: FHT_g, moving x -> xm_k [(i',h), tok] per i-block k
            xm = mpool.tile([128, KT * ST], DTB, tag="m1")
            for k in range(KT):
                ps = ps1pool.tile([128, ST], DTF, tag="ps1")
                nc.tensor.matmul(
                    ps[:], lhsT=hmt[:], rhs=xa[:, k * ST : (k + 1) * ST],
                    start=True, stop=True,
                )
                drain(k, xm[:, k * ST : (k + 1) * ST], ps[:])

            # ---- R1: regroup -> xb_h [i, tok] (h-grouped)
            xb = mpool.tile([128, KT * ST], DTB, tag="m2")
            regroup(xb, xm, xmd[st])

            # ---- P2: grouped ternary matmul -> yp_h [o, tok]
            yp = mpool.tile([128, KT * ST], DTB, tag="m3")
            for h in range(KT):
                ps = ps2pool.tile([128, ST], DTF, tag="ps2")
                nc.tensor.matmul(
                    ps[:], lhsT=wqt[:, h * 128 : (h + 1) * 128],
                    rhs=xb[:, h * ST : (h + 1) * ST],
                    start=True, stop=True,
                )
                drain(h + 1, yp[:, h * ST : (h + 1) * ST], ps[:])

            # ---- R2: regroup -> ya_m [(o',h), tok] per o-block m
            ya = mpool.tile([128, KT * ST], DTB, tag="m4")
            regroup(ya, yp, ypd[st])

            # ---- P3: FHT_h -> [(g,o'), tok]; drain with beta scale, bf16
            yf = mpool.tile([128, KT * ST], DTB, tag="m2")
            for m in range(KT):
                ps = ps3pool.tile([128, ST], DTF, tag="ps3")
                nc.tensor.matmul(
                    ps[:], lhsT=hmt2[:], rhs=ya[:, m * ST : (m + 1) * ST],
                    start=True, stop=True,
                )
                if m % 2 == 0:
                    nc.scalar.activation(
                        yf[:, m * ST : (m + 1) * ST], ps[:],
                        mybir.ActivationFunctionType.Copy,
                        scale=bct[:, m : m + 1],
                    )
                else:
                    nc.vector.tensor_scalar_mul(
                        yf[:, m * ST : (m + 1) * ST], ps[:], bct[:, m : m + 1]
                    )
            nc.sync.dma_start(yout[st][:, :HF], yf[:, :HF])
            nc.scalar.dma_start(yout[st][:, HF:], yf[:, HF:])

        if loop_r == 1:
            for st in range(NST):
                supertile(st)
        else:
            with tc.For_i(0, loop_r, 1):
                for st in range(NST):
                    supertile(st)


@functools.lru_cache(maxsize=4)
def build_program(loop_r=1):
    nc = bacc.Bacc("TRN2", target_bir_lowering=False, debug=False)
    xin = nc.dram_tensor("xin", [NST, 128, KT * ST], DTB, kind="ExternalInput").ap()
    hm = nc.dram_tensor("hmat", [2, 128, 128], DTB, kind="ExternalInput").ap()
    wqm = nc.dram_tensor("wqm", [128, G * OO], DTB, kind="ExternalInput").ap()
    bc = nc.dram_tensor("betacol", [128, G], DTF, kind="ExternalInput").ap()
    yout = nc.dram_tensor("yout", [NST, 128, KT * ST], DTB, kind="ExternalOutput").ap()
    xmd = nc.dram_tensor("xmd", [NST, 128, KT * ST], DTB, kind="Internal").ap()
    ypd = nc.dram_tensor("ypd", [NST, 128, KT * ST], DTB, kind="Internal").ap()
    with tile.TileContext(nc) as tc:
        build_body(nc, tc, xin, hm, wqm, bc, yout, xmd, ypd, loop_r=loop_r)
    nc.compile()
    return nc


def host_prep(x, weight, alpha, beta):
    """Returns per-core input maps. Pure f32 numpy glue + bf16 casts."""
    H = _hadamard(G)  # [g,h] +-1

    w = np.asarray(weight, dtype=np.float32)
    scale = np.float32(np.mean(np.abs(w))) + np.float32(1e-8)
    wq3 = np.clip(np.round(w / scale), -1.0, 1.0).astype(np.float32)  # [h,o,i]

    # x * alpha, feature order f' = i*32+g (i-major)
    xp = np.asarray(x, dtype=np.float32).reshape(BT, G, IO) * np.asarray(
        alpha, dtype=np.float32
    )[None]
    xp = np.ascontiguousarray(xp.transpose(0, 2, 1)).reshape(BT, D)  # [tok, f']
    # device layout: [core, st, p(128), k(32), t(512)] with f' = k*128 + p
    xin_all = np.ascontiguousarray(
        xp.reshape(NCORES, NST, ST, KT, 128).transpose(0, 1, 4, 3, 2)
    ).reshape(NCORES, NST, 128, KT * ST).astype(BF16)

    hmat = np.stack(
        [
            np.kron(np.eye(4, dtype=np.float32), H),  # P1: I4 (x) H
            np.kron(H, np.eye(4, dtype=np.float32)),  # P3: H (x) I4
        ]
    ).astype(BF16)
    # wq_sb[i, h*128 + o'*32 + m] = wq3[h, 4m+o', i]  (o = 4m+o')
    wq_sb = np.ascontiguousarray(
        wq3.reshape(G, G, 4, IO).transpose(3, 0, 2, 1)
    ).reshape(IO, G * OO).astype(BF16)

    beta_f = np.asarray(beta, dtype=np.float32) * (scale / np.float32(G))  # [g,o]
    # betacol[p = 4g+o', m] = beta_f[g, 4m+o']
    bct = np.ascontiguousarray(
        beta_f.reshape(G, G, 4).transpose(0, 2, 1)
    ).reshape(128, G).astype(np.float32)

    in_maps = []
    for c in range(NCORES):
        in_maps.append(
            {
                "xin": xin_all[c],
                "hmat": hmat,
                "wqm": wq_sb,
                "betacol": bct,
            }
        )
    return in_maps


def host_post(results):
    ydev = np.stack([r["yout"] for r in results])  # [8, NST, 128, 16384] bf16
    # [c, st, p=(4g+o'), m*512+t] -> y[tok, g*128+4m+o']
    y = ydev.reshape(NCORES, NST, G, 4, KT, ST)  # [c, st, g, o', m, t]
    y = y.transpose(0, 1, 5, 2, 4, 3)  # [c, st, t, g, m, o']
    y = np.ascontiguousarray(y).astype(np.float32).reshape(BT, D)
    return y.reshape(B, T, D)


def kernel(x, weight, alpha, beta):
    nc = build_program(loop_r=1)
    in_maps = host_prep(x, weight, alpha, beta)
    res = run_bass_kernel_spmd(nc, in_maps, core_ids=list(range(NCORES)))
    return host_post(res.results)
